# revision 22
# baseline (speedup 1.0000x reference)
"""ClusterMemory (scatter_memory) Trainium2 kernel, 8-core SPMD.

Problem (B=2048, D=256, N=65536):
  xn      = inputs / max(||inputs||_row, 1e-12)
  outputs = (xn @ features.T) / 0.05                      # [B, N] logits
  new_features = sequential momentum-EMA scatter update:
      for i in range(B): y = targets[i]
          f = 0.2*feats[y] + 0.8*xn[i]; feats[y] = f/||f||

Sharding: features / logits are column-sharded over num_samples (N) across
8 cores; inputs replicated. Each core:
  - computes xn on device and the [2048, 8192] logits shard via a bf16
    tensor-engine matmul (fp32 accumulate), streaming the 64 MB fp32 shard
    out to HBM (memory-bound).
  - applies the EMA updates for rows it owns.  Write-after-write chains on
    duplicate targets are handled with a "rounds" decomposition: round r
    applies the (r+1)-th update of every row with multiplicity > r.  Rows
    are sorted by multiplicity (host side) so each round is a prefix of the
    previous round's result tile; rounds become static prefix-slices with
    per-partition {0.2,1.0}/{0.8,0.0} coefficient vectors masking the tail.
    All EMA/normalize math runs on device in fp32; the host only routes rows
    to their owning shard and scatters the returned rows back (gather /
    unshard).
"""

import os
import sys

import numpy as np

if "/opt/trn_rl_repo" not in sys.path:
    sys.path.insert(0, "/opt/trn_rl_repo")

# ---- problem constants (hardcoded per contract) ----
B, D, N = 2048, 256, 65536
M = 8                    # cores
SH = N // M              # 8192 rows of features per core
TEMP = 0.05
MOM = 0.2
EPS = 1e-12

P = 128                  # partitions
CAP0 = 384               # round-0 capacity (distinct owned rows), 3 tiles
CAPR = 128               # capacity of rounds 1..ROUNDS-1 (1 tile each)
ROUNDS = 4               # max update-chain length handled on device
                         # (overflow rows fall back to exact host numpy)
XR_ROWS = CAP0 + (ROUNDS - 1) * CAPR   # 1152 routed-x rows
NB = B // P              # 16 b-chunks
NN = SH // 512           # 16 n-chunks of 512
KD = D // P              # 2 contraction chunks

_NC = None               # cached Bass program
LAST_EXEC_NS = None      # populated when BASS_TRACE=1


def _build_nc():
    import concourse.bacc as bacc
    import concourse.tile as tile
    from concourse import mybir
    from concourse.masks import make_identity

    f32 = mybir.dt.float32
    bf16 = mybir.dt.bfloat16
    AF = mybir.ActivationFunctionType

    nc = bacc.Bacc(None)

    x_d = nc.dram_tensor("x", [B, D], f32, kind="ExternalInput")
    ftT_d = nc.dram_tensor("ftT", [D, SH], bf16, kind="ExternalInput")
    xr_d = nc.dram_tensor("xr", [XR_ROWS, D], f32, kind="ExternalInput")
    g0_d = nc.dram_tensor("g0", [CAP0, D], f32, kind="ExternalInput")
    coef_d = nc.dram_tensor("coef", [P, 2 * (ROUNDS - 1)], f32, kind="ExternalInput")

    out_d = nc.dram_tensor("out", [B, SH], f32, kind="ExternalOutput")
    upd_d = nc.dram_tensor("upd", [CAP0, D], f32, kind="ExternalOutput")

    with tile.TileContext(nc) as tc:
        with (
            tc.tile_pool(name="singles", bufs=1) as singles,
            tc.tile_pool(name="xg", bufs=3) as xgp,
            tc.tile_pool(name="xnb", bufs=3) as xnbp,
            tc.tile_pool(name="small", bufs=8) as small,
            tc.tile_pool(name="scratch", bufs=2) as scratch,
            tc.tile_pool(name="stage", bufs=4) as stagep,
            tc.tile_pool(name="psum", bufs=2, space="PSUM") as psump,
        ):
            # ---------- static loads ----------
            # Two DMA rings in parallel: x / routed-update tensors on the
            # HWDGE (sync) ring, featT quarters on the SWDGE (gpsimd) ring,
            # ordered so the first matmuls (q-major loop) unblock in a few us.
            ident = singles.tile([P, P], bf16, tag="ident")
            make_identity(nc, ident[:])

            ft = []
            for k in range(KD):
                t = singles.tile([P, SH], bf16, tag=f"ft{k}", name=f"ft{k}")
                ft.append(t)
            for q in range(NN // 4):
                for k in range(KD):
                    nc.gpsimd.dma_start(
                        out=ft[k][:, q * 2048:(q + 1) * 2048],
                        in_=ftT_d[k * P:(k + 1) * P, q * 2048:(q + 1) * 2048],
                    )

            xnT = [
                singles.tile([P, B], bf16, tag=f"xnT{k}", name=f"xnT{k}")
                for k in range(KD)
            ]

            xrt = singles.tile([P, XR_ROWS // P, D], f32, tag="xrt")
            g0t = singles.tile([P, CAP0 // P, D], f32, tag="g0t")
            coefT = singles.tile([P, 2 * (ROUNDS - 1)], f32, tag="coef")

            def rownorm_recip(src_ap, with_eps):
                """1/max(||row||, eps) as a [P, 1] f32 tile (eps optional)."""
                sc = scratch.tile([P, D], f32, tag="scr")
                s = small.tile([P, 1], f32, tag="s")
                nc.scalar.activation(
                    out=sc[:], in_=src_ap, func=AF.Square, accum_out=s[:]
                )
                r = small.tile([P, 1], f32, tag="r")
                nc.scalar.activation(out=r[:], in_=s[:], func=AF.Sqrt)
                if with_eps:
                    nc.vector.tensor_scalar_max(out=r[:], in0=r[:], scalar1=EPS)
                nc.vector.reciprocal(out=r[:], in_=r[:])
                return r

            # ---------- xn = normalize(x) * (1/TEMP), transposed to [D, B] ----------
            for g in range(NB // 4):
                xg = xgp.tile([P, 4, D], f32, tag="xg")
                nc.sync.dma_start(
                    out=xg[:],
                    in_=x_d[g * 4 * P:(g + 1) * 4 * P, :].rearrange(
                        "(t p) d -> p t d", p=P
                    ),
                )
                for t in range(4):
                    i = g * 4 + t
                    r = rownorm_recip(xg[:, t, :], with_eps=True)
                    xnb = xnbp.tile([P, D], bf16, tag="xnb")
                    nc.vector.tensor_scalar(
                        out=xnb[:],
                        in0=xg[:, t, :],
                        scalar1=r[:],
                        scalar2=1.0 / TEMP,
                        op0=mybir.AluOpType.mult,
                        op1=mybir.AluOpType.mult,
                    )
                    for k in range(KD):
                        pt = psump.tile([P, P], bf16, tag="ps", name=f"pt{i}_{k}")
                        nc.tensor.transpose(
                            out=pt[:], in_=xnb[:, k * P:(k + 1) * P], identity=ident[:]
                        )
                        nc.vector.tensor_copy(
                            out=xnT[k][:, i * P:(i + 1) * P], in_=pt[:]
                        )

            # update-path loads (cheap; compute is emitted after the matmul
            # loop so it fills engine idle slots / the DMA drain tail
            # instead of stalling PSUM evacuation mid-stream)
            nc.sync.dma_start(
                out=xrt[:], in_=xr_d[:].rearrange("(t p) d -> p t d", p=P)
            )
            nc.sync.dma_start(
                out=g0t[:], in_=g0_d[:].rearrange("(t p) d -> p t d", p=P)
            )
            nc.sync.dma_start(out=coefT[:], in_=coef_d[:])

            # ---------- logits matmul: out[b, n] = sum_d xnT[d, b] * ft[d, n] ----------
            # One 4-bank PSUM tile per (b-chunk, quarter); evacuated with a
            # single [128, 2048] copy alternating between DVE and ACT so the
            # two engines share the PSUM-drain load.
            AF2 = mybir.ActivationFunctionType
            for q in range(NN // 4):
                for bi in range(NB):
                    pq = psump.tile([P, 2048], f32, tag="ps", name=f"pq{bi}_{q}")
                    for k in range(KD):
                        for j in range(4):
                            n0 = (q * 4 + j) * 512
                            nc.tensor.matmul(
                                out=pq[:, j * 512:(j + 1) * 512],
                                lhsT=xnT[k][:, bi * P:(bi + 1) * P],
                                rhs=ft[k][:, n0:n0 + 512],
                                start=(k == 0),
                                stop=(k == KD - 1),
                            )
                    stg = stagep.tile([P, 2048], f32, tag="stg")
                    if (q * NB + bi) % 2 == 0:
                        nc.scalar.activation(out=stg[:], in_=pq[:], func=AF2.Copy)
                    else:
                        nc.vector.tensor_copy(out=stg[:], in_=pq[:])
                    nc.sync.dma_start(
                        out=out_d[bi * P:(bi + 1) * P, q * 2048:(q + 1) * 2048],
                        in_=stg[:],
                    )

            # ---------- EMA scatter update (rounds) ----------
            # All [P, D] elementwise work runs on GpSimd (SBUF-only, mostly
            # idle) so the update chain never steals DVE/ACT cycles from the
            # PSUM-evacuation stream; only tiny [P, 1] sqrt/recip ops touch
            # ACT/DVE.
            def rownorm_recip_gp(src_ap, with_eps):
                # ACT Square+accum for sumsq (small), everything [P, 1] tiny
                sc = scratch.tile([P, D], f32, tag="scr")
                s = small.tile([P, 1], f32, tag="s")
                nc.scalar.activation(
                    out=sc[:], in_=src_ap, func=AF.Square, accum_out=s[:]
                )
                r = small.tile([P, 1], f32, tag="r")
                nc.scalar.activation(out=r[:], in_=s[:], func=AF.Sqrt)
                if with_eps:
                    nc.gpsimd.tensor_scalar_max(out=r[:], in0=r[:], scalar1=EPS)
                nc.vector.reciprocal(out=r[:], in_=r[:])
                return r

            # normalize routed x rows (with eps guard, matching reference xn)
            for t in range(XR_ROWS // P):
                r = rownorm_recip_gp(xrt[:, t, :], with_eps=True)
                nc.gpsimd.tensor_scalar_mul(
                    out=xrt[:, t, :], in0=xrt[:, t, :], scalar1=r[:]
                )

            def ema_step(g_ap, x_ap, a, b):
                """g = normalize(a*g + b*x); a/b immediates or [P,1] APs."""
                t1 = small.tile([P, D], f32, tag="t1")
                t2 = small.tile([P, D], f32, tag="t2")
                nc.gpsimd.tensor_scalar_mul(out=t1[:], in0=g_ap, scalar1=a)
                nc.gpsimd.tensor_scalar_mul(out=t2[:], in0=x_ap, scalar1=b)
                nc.gpsimd.tensor_add(out=g_ap, in0=t1[:], in1=t2[:])
                r = rownorm_recip_gp(g_ap, with_eps=False)
                nc.gpsimd.tensor_scalar_mul(out=g_ap, in0=g_ap, scalar1=r[:])

            for t in range(CAP0 // P):       # round 0: all entries
                ema_step(g0t[:, t, :], xrt[:, t, :], MOM, 1.0 - MOM)
            for rr in range(1, ROUNDS):      # rounds 1..: prefix tile 0
                a = coefT[:, 2 * (rr - 1):2 * (rr - 1) + 1]
                b = coefT[:, 2 * (rr - 1) + 1:2 * (rr - 1) + 2]
                ema_step(g0t[:, 0, :], xrt[:, CAP0 // P + (rr - 1), :], a, b)

            nc.sync.dma_start(
                out=upd_d[:].rearrange("(t p) d -> p t d", p=P), in_=g0t[:]
            )

    nc.finalize()   # Bacc: register allocation + codegen passes
    return nc


def _get_nc():
    global _NC
    if _NC is None:
        _NC = _build_nc()
    return _NC


def _np_chain(feat_row, xs):
    """Exact fp32 EMA chain for host-fallback rows."""
    f = feat_row.astype(np.float32).copy()
    for xv in xs:
        f = np.float32(MOM) * f + np.float32(1.0 - MOM) * xv
        f = f / np.float32(np.linalg.norm(f))
    return f


def _ensure_axon_hooks():
    """bass_utils imports antenv.axon_hooks when BASS_TRACE is set; some
    images lack that module.  Provide a no-op fallback (tracing degrades
    gracefully) without shadowing a real one."""
    try:
        import antenv.axon_hooks  # noqa: F401
    except ImportError:
        import types
        import antenv

        mod = types.ModuleType("antenv.axon_hooks")
        _hook = [None]
        mod.set_axon_ntff_profile_hook = lambda h: _hook.__setitem__(0, h)
        mod.get_axon_ntff_profile_hook = lambda: _hook[0]
        sys.modules["antenv.axon_hooks"] = mod
        antenv.axon_hooks = mod


def kernel(inputs, targets, features):
    global LAST_EXEC_NS
    _ensure_axon_hooks()
    from concourse.bass_utils import run_bass_kernel_spmd

    x = np.ascontiguousarray(inputs, dtype=np.float32)
    feat = np.ascontiguousarray(features, dtype=np.float32)
    tgt = np.asarray(targets).astype(np.int64)

    try:
        import ml_dtypes
        bf16 = ml_dtypes.bfloat16
    except ImportError:  # pragma: no cover
        from jax import numpy as jnp
        bf16 = jnp.bfloat16

    featT_b = np.ascontiguousarray(feat.T.astype(bf16))     # [D, N] bf16

    owner = tgt // SH
    in_maps = []
    core_rows = []      # per core: list of device global rows (sorted order)
    host_rows = []      # per core: list of (row, [b indices]) handled on host
    for c in range(M):
        sel = np.nonzero(owner == c)[0]
        groups = {}
        for b in sel:
            groups.setdefault(int(tgt[b]), []).append(int(b))
        rows_sorted = sorted(groups.keys(), key=lambda rr: -len(groups[rr]))
        dev, host = [], []
        for rr in rows_sorted:
            mlt = len(groups[rr])
            i = len(dev)
            if mlt > ROUNDS or (mlt >= 2 and i >= CAPR) or i >= CAP0:
                host.append((rr, groups[rr]))
            else:
                dev.append(rr)
        k0 = len(dev)
        xr = np.ones((XR_ROWS, D), dtype=np.float32)
        g0 = np.ones((CAP0, D), dtype=np.float32)
        coef = np.empty((P, 2 * (ROUNDS - 1)), dtype=np.float32)
        if k0:
            g0[:k0] = feat[dev]
            xr[:k0] = x[[groups[rr][0] for rr in dev]]
        for rr_i in range(1, ROUNDS):
            kr = sum(1 for rr in dev if len(groups[rr]) > rr_i)
            for i in range(kr):
                xr[CAP0 + (rr_i - 1) * CAPR + i] = x[groups[dev[i]][rr_i]]
            coef[:, 2 * (rr_i - 1)] = np.where(np.arange(P) < kr, MOM, 1.0)
            coef[:, 2 * (rr_i - 1) + 1] = np.where(np.arange(P) < kr, 1.0 - MOM, 0.0)
        core_rows.append(dev)
        host_rows.append(host)
        in_maps.append({
            "x": x,
            "ftT": np.ascontiguousarray(featT_b[:, c * SH:(c + 1) * SH]),
            "xr": xr,
            "g0": g0,
            "coef": coef,
        })

    nc = _get_nc()
    res = run_bass_kernel_spmd(nc, in_maps, list(range(M)))
    LAST_EXEC_NS = res.exec_time_ns

    outputs = np.concatenate([res.results[c]["out"] for c in range(M)], axis=1)

    new_features = feat.copy()
    xn_cache = {}

    def xn_row(b):
        if b not in xn_cache:
            nrm = max(float(np.linalg.norm(x[b])), EPS)
            xn_cache[b] = (x[b] / np.float32(nrm)).astype(np.float32)
        return xn_cache[b]

    for c in range(M):
        dev = core_rows[c]
        if dev:
            new_features[dev] = res.results[c]["upd"][:len(dev)]
        for rr, bs in host_rows[c]:
            new_features[rr] = _np_chain(feat[rr], [xn_row(b) for b in bs])

    return outputs, new_features


# revision 24
# speedup vs baseline: 1.3227x; 1.3227x over previous
"""ClusterMemory (scatter_memory) Trainium2 kernel, 8-core SPMD.

Problem (B=2048, D=256, N=65536):
  xn      = inputs / max(||inputs||_row, 1e-12)
  outputs = (xn @ features.T) / 0.05                      # [B, N] logits
  new_features = sequential momentum-EMA scatter update:
      for i in range(B): y = targets[i]
          f = 0.2*feats[y] + 0.8*xn[i]; feats[y] = f/||f||

Sharding: features / logits are column-sharded over num_samples (N) across
8 cores; inputs replicated. Each core:
  - computes xn on device and the [2048, 8192] logits shard via a bf16
    tensor-engine matmul (fp32 accumulate), streaming the 64 MB fp32 shard
    out to HBM (memory-bound).
  - applies the EMA updates for rows it owns.  Write-after-write chains on
    duplicate targets are handled with a "rounds" decomposition: round r
    applies the (r+1)-th update of every row with multiplicity > r.  Rows
    are sorted by multiplicity (host side) so each round is a prefix of the
    previous round's result tile; rounds become static prefix-slices with
    per-partition {0.2,1.0}/{0.8,0.0} coefficient vectors masking the tail.
    All EMA/normalize math runs on device in fp32; the host only routes rows
    to their owning shard and scatters the returned rows back (gather /
    unshard).
"""

import os
import sys

import numpy as np

if "/opt/trn_rl_repo" not in sys.path:
    sys.path.insert(0, "/opt/trn_rl_repo")

# ---- problem constants (hardcoded per contract) ----
B, D, N = 2048, 256, 65536
M = 8                    # cores
SH = N // M              # 8192 rows of features per core
TEMP = 0.05
MOM = 0.2
EPS = 1e-12

P = 128                  # partitions
CAP0 = 384               # round-0 capacity (distinct owned rows), 3 tiles
CAPR = 128               # capacity of rounds 1..ROUNDS-1 (1 tile each)
ROUNDS = 4               # max update-chain length handled on device
                         # (overflow rows fall back to exact host numpy)
XR_ROWS = CAP0 + (ROUNDS - 1) * CAPR   # 1152 routed-x rows
NB = B // P              # 16 b-chunks
NN = SH // 512           # 16 n-chunks of 512
KD = D // P              # 2 contraction chunks

_NC = None               # cached Bass program
LAST_EXEC_NS = None      # populated when BASS_TRACE=1


def _build_nc():
    import concourse.bacc as bacc
    import concourse.tile as tile
    from concourse import mybir
    from concourse.masks import make_identity

    f32 = mybir.dt.float32
    bf16 = mybir.dt.bfloat16
    AF = mybir.ActivationFunctionType

    nc = bacc.Bacc(None)

    x_d = nc.dram_tensor("x", [B, D], f32, kind="ExternalInput")
    ftT_d = nc.dram_tensor("ftT", [D, SH], bf16, kind="ExternalInput")
    xr_d = nc.dram_tensor("xr", [XR_ROWS, D], f32, kind="ExternalInput")
    g0_d = nc.dram_tensor("g0", [CAP0, D], f32, kind="ExternalInput")
    coef_d = nc.dram_tensor("coef", [P, 2 * (ROUNDS - 1)], f32, kind="ExternalInput")

    out_d = nc.dram_tensor("out", [B, SH], f32, kind="ExternalOutput")
    upd_d = nc.dram_tensor("upd", [CAP0, D], f32, kind="ExternalOutput")

    with tile.TileContext(nc) as tc:
        with (
            tc.tile_pool(name="singles", bufs=1) as singles,
            tc.tile_pool(name="xg", bufs=3) as xgp,
            tc.tile_pool(name="xnb", bufs=3) as xnbp,
            tc.tile_pool(name="small", bufs=8) as small,
            tc.tile_pool(name="scratch", bufs=2) as scratch,
            tc.tile_pool(name="stage", bufs=4) as stagep,
            tc.tile_pool(name="psum", bufs=2, space="PSUM") as psump,
        ):
            # ---------- static loads ----------
            # Two DMA rings in parallel: x / routed-update tensors on the
            # HWDGE (sync) ring, featT quarters on the SWDGE (gpsimd) ring,
            # ordered so the first matmuls (q-major loop) unblock in a few us.
            ident = singles.tile([P, P], bf16, tag="ident")
            make_identity(nc, ident[:])

            ft = []
            for k in range(KD):
                t = singles.tile([P, SH], bf16, tag=f"ft{k}", name=f"ft{k}")
                ft.append(t)
            for q in range(NN // 4):
                for k in range(KD):
                    nc.gpsimd.dma_start(
                        out=ft[k][:, q * 2048:(q + 1) * 2048],
                        in_=ftT_d[k * P:(k + 1) * P, q * 2048:(q + 1) * 2048],
                    )

            xnT = [
                singles.tile([P, B], bf16, tag=f"xnT{k}", name=f"xnT{k}")
                for k in range(KD)
            ]

            xrt = singles.tile([P, XR_ROWS // P, D], f32, tag="xrt")
            g0t = singles.tile([P, CAP0 // P, D], f32, tag="g0t")
            coefT = singles.tile([P, 2 * (ROUNDS - 1)], f32, tag="coef")

            def rownorm_recip(src_ap, with_eps):
                """1/max(||row||, eps) as a [P, 1] f32 tile (eps optional)."""
                sc = scratch.tile([P, D], f32, tag="scr")
                s = small.tile([P, 1], f32, tag="s")
                nc.scalar.activation(
                    out=sc[:], in_=src_ap, func=AF.Square, accum_out=s[:]
                )
                r = small.tile([P, 1], f32, tag="r")
                nc.scalar.activation(out=r[:], in_=s[:], func=AF.Sqrt)
                if with_eps:
                    nc.vector.tensor_scalar_max(out=r[:], in0=r[:], scalar1=EPS)
                nc.vector.reciprocal(out=r[:], in_=r[:])
                return r

            # ---------- xn = normalize(x) * (1/TEMP), transposed to [D, B] ----------
            for g in range(NB // 4):
                xg = xgp.tile([P, 4, D], f32, tag="xg")
                nc.sync.dma_start(
                    out=xg[:],
                    in_=x_d[g * 4 * P:(g + 1) * 4 * P, :].rearrange(
                        "(t p) d -> p t d", p=P
                    ),
                )
                for t in range(4):
                    i = g * 4 + t
                    r = rownorm_recip(xg[:, t, :], with_eps=True)
                    xnb = xnbp.tile([P, D], bf16, tag="xnb")
                    nc.vector.tensor_scalar(
                        out=xnb[:],
                        in0=xg[:, t, :],
                        scalar1=r[:],
                        scalar2=1.0 / TEMP,
                        op0=mybir.AluOpType.mult,
                        op1=mybir.AluOpType.mult,
                    )
                    for k in range(KD):
                        pt = psump.tile([P, P], bf16, tag="ps", name=f"pt{i}_{k}")
                        nc.tensor.transpose(
                            out=pt[:], in_=xnb[:, k * P:(k + 1) * P], identity=ident[:]
                        )
                        nc.vector.tensor_copy(
                            out=xnT[k][:, i * P:(i + 1) * P], in_=pt[:]
                        )

            # ---------- EMA scatter update, staggered through the stream ----
            # The update chain is deep, so the scheduler wants to run it as
            # one burst that starves PSUM evacuation (DVE+ACT).  Instead we
            # dole it out in 6 small pieces gated by their xr-slice loads,
            # which are emitted between store quarters on the same sync DMA
            # ring (FIFO) - each piece becomes ready only mid-stream.
            def ema_step(g_ap, x_ap, a, b):
                """g = normalize(a*g + b*x); a/b immediates or [P,1] APs."""
                t1 = small.tile([P, D], f32, tag="t1")
                t2 = small.tile([P, D], f32, tag="t2")
                nc.vector.tensor_scalar_mul(out=t1[:], in0=g_ap, scalar1=a)
                nc.vector.tensor_scalar_mul(out=t2[:], in0=x_ap, scalar1=b)
                nc.vector.tensor_add(out=g_ap, in0=t1[:], in1=t2[:])
                r = rownorm_recip(g_ap, with_eps=False)
                nc.vector.tensor_scalar_mul(out=g_ap, in0=g_ap, scalar1=r[:])

            def emit_update_piece(t):
                if t == 0:
                    nc.sync.dma_start(
                        out=g0t[:], in_=g0_d[:].rearrange("(t p) d -> p t d", p=P)
                    )
                    nc.sync.dma_start(out=coefT[:], in_=coef_d[:])
                nc.sync.dma_start(
                    out=xrt[:, t, :], in_=xr_d[t * P:(t + 1) * P, :]
                )
                r = rownorm_recip(xrt[:, t, :], with_eps=True)
                nc.vector.tensor_scalar_mul(
                    out=xrt[:, t, :], in0=xrt[:, t, :], scalar1=r[:]
                )
                nt0 = CAP0 // P
                if t < nt0:                  # round 0 on tile t
                    ema_step(g0t[:, t, :], xrt[:, t, :], MOM, 1.0 - MOM)
                else:                        # round (t-nt0+1) on tile 0 prefix
                    rr = t - nt0 + 1
                    a = coefT[:, 2 * (rr - 1):2 * (rr - 1) + 1]
                    b = coefT[:, 2 * (rr - 1) + 1:2 * (rr - 1) + 2]
                    ema_step(g0t[:, 0, :], xrt[:, t, :], a, b)

            # ---------- logits matmul: out[b, n] = sum_d xnT[d, b] * ft[d, n] ----------
            # One 4-bank PSUM tile per (b-chunk, quarter); evacuated with a
            # single [128, 2048] copy alternating between DVE and ACT so the
            # two engines share the PSUM-drain load.
            AF2 = mybir.ActivationFunctionType
            for q in range(NN // 4):
                for bi in range(NB):
                    pq = psump.tile([P, 2048], f32, tag="ps", name=f"pq{bi}_{q}")
                    for k in range(KD):
                        for j in range(4):
                            n0 = (q * 4 + j) * 512
                            nc.tensor.matmul(
                                out=pq[:, j * 512:(j + 1) * 512],
                                lhsT=xnT[k][:, bi * P:(bi + 1) * P],
                                rhs=ft[k][:, n0:n0 + 512],
                                start=(k == 0),
                                stop=(k == KD - 1),
                            )
                    stg = stagep.tile([P, 2048], f32, tag="stg")
                    if (q * NB + bi) % 2 == 0:
                        nc.scalar.activation(out=stg[:], in_=pq[:], func=AF2.Copy)
                    else:
                        nc.vector.tensor_copy(out=stg[:], in_=pq[:])
                    nc.sync.dma_start(
                        out=out_d[bi * P:(bi + 1) * P, q * 2048:(q + 1) * 2048],
                        in_=stg[:],
                    )
                    qi = q * NB + bi
                    if qi >= 12 and (qi - 12) % 8 == 0 and (qi - 12) // 8 < XR_ROWS // P:
                        emit_update_piece((qi - 12) // 8)

            nc.sync.dma_start(
                out=upd_d[:].rearrange("(t p) d -> p t d", p=P), in_=g0t[:]
            )

    nc.finalize()   # Bacc: register allocation + codegen passes
    return nc


def _get_nc():
    global _NC
    if _NC is None:
        _NC = _build_nc()
    return _NC


def _np_chain(feat_row, xs):
    """Exact fp32 EMA chain for host-fallback rows."""
    f = feat_row.astype(np.float32).copy()
    for xv in xs:
        f = np.float32(MOM) * f + np.float32(1.0 - MOM) * xv
        f = f / np.float32(np.linalg.norm(f))
    return f


def _ensure_axon_hooks():
    """bass_utils imports antenv.axon_hooks when BASS_TRACE is set; some
    images lack that module.  Provide a no-op fallback (tracing degrades
    gracefully) without shadowing a real one."""
    try:
        import antenv.axon_hooks  # noqa: F401
    except ImportError:
        import types
        import antenv

        mod = types.ModuleType("antenv.axon_hooks")
        _hook = [None]
        mod.set_axon_ntff_profile_hook = lambda h: _hook.__setitem__(0, h)
        mod.get_axon_ntff_profile_hook = lambda: _hook[0]
        sys.modules["antenv.axon_hooks"] = mod
        antenv.axon_hooks = mod


def kernel(inputs, targets, features):
    global LAST_EXEC_NS
    _ensure_axon_hooks()
    from concourse.bass_utils import run_bass_kernel_spmd

    x = np.ascontiguousarray(inputs, dtype=np.float32)
    feat = np.ascontiguousarray(features, dtype=np.float32)
    tgt = np.asarray(targets).astype(np.int64)

    try:
        import ml_dtypes
        bf16 = ml_dtypes.bfloat16
    except ImportError:  # pragma: no cover
        from jax import numpy as jnp
        bf16 = jnp.bfloat16

    featT_b = np.ascontiguousarray(feat.T.astype(bf16))     # [D, N] bf16

    owner = tgt // SH
    in_maps = []
    core_rows = []      # per core: list of device global rows (sorted order)
    host_rows = []      # per core: list of (row, [b indices]) handled on host
    for c in range(M):
        sel = np.nonzero(owner == c)[0]
        groups = {}
        for b in sel:
            groups.setdefault(int(tgt[b]), []).append(int(b))
        rows_sorted = sorted(groups.keys(), key=lambda rr: -len(groups[rr]))
        dev, host = [], []
        for rr in rows_sorted:
            mlt = len(groups[rr])
            i = len(dev)
            if mlt > ROUNDS or (mlt >= 2 and i >= CAPR) or i >= CAP0:
                host.append((rr, groups[rr]))
            else:
                dev.append(rr)
        k0 = len(dev)
        xr = np.ones((XR_ROWS, D), dtype=np.float32)
        g0 = np.ones((CAP0, D), dtype=np.float32)
        coef = np.empty((P, 2 * (ROUNDS - 1)), dtype=np.float32)
        if k0:
            g0[:k0] = feat[dev]
            xr[:k0] = x[[groups[rr][0] for rr in dev]]
        for rr_i in range(1, ROUNDS):
            kr = sum(1 for rr in dev if len(groups[rr]) > rr_i)
            for i in range(kr):
                xr[CAP0 + (rr_i - 1) * CAPR + i] = x[groups[dev[i]][rr_i]]
            coef[:, 2 * (rr_i - 1)] = np.where(np.arange(P) < kr, MOM, 1.0)
            coef[:, 2 * (rr_i - 1) + 1] = np.where(np.arange(P) < kr, 1.0 - MOM, 0.0)
        core_rows.append(dev)
        host_rows.append(host)
        in_maps.append({
            "x": x,
            "ftT": np.ascontiguousarray(featT_b[:, c * SH:(c + 1) * SH]),
            "xr": xr,
            "g0": g0,
            "coef": coef,
        })

    nc = _get_nc()
    res = run_bass_kernel_spmd(nc, in_maps, list(range(M)))
    LAST_EXEC_NS = res.exec_time_ns

    outputs = np.concatenate([res.results[c]["out"] for c in range(M)], axis=1)

    new_features = feat.copy()
    xn_cache = {}

    def xn_row(b):
        if b not in xn_cache:
            nrm = max(float(np.linalg.norm(x[b])), EPS)
            xn_cache[b] = (x[b] / np.float32(nrm)).astype(np.float32)
        return xn_cache[b]

    for c in range(M):
        dev = core_rows[c]
        if dev:
            new_features[dev] = res.results[c]["upd"][:len(dev)]
        for rr, bs in host_rows[c]:
            new_features[rr] = _np_chain(feat[rr], [xn_row(b) for b in bs])

    return outputs, new_features
